# revision 16
# baseline (speedup 1.0000x reference)
"""Trainium2 Bass kernel for the MoE-routing branch module.

Per sample: 8 expert MLPs (512->512 relu, 512->1000), softmax confidence
per expert, greedy diversity-based expert selection, temperature-softmax
gates over selected experts, gated combination of expert logits.

Sharding: data-parallel over batch B=8192 across 8 cores (1024 samples
each); expert weights replicated.

Matmul precision: fp32 operands are split hi/lo into bf16 on the host and
the products are computed with 3 bf16 matmul passes (hi*hi + hi*lo +
lo*hi), accumulating in fp32 PSUM. Residual error ~2^-18 per term keeps
the greedy selection bit-faithful to the fp32 reference (validated: the
selection tolerates logit perturbations up to ~1e-4; this scheme gives
~2e-6).
"""

import os
import numpy as np
import ml_dtypes
from contextlib import ExitStack

B, D, C, K = 8192, 512, 1000, 8
N_CORES = 8
P = 128
BC = B // N_CORES            # samples per core
NT = BC // P                 # sample tiles per core
CHUNK_T = 2                  # tiles per weight-streaming chunk
QD = D // P                  # contraction chunks (4)
MD = D // P                  # layer-1 output chunks (4)
BIG = 10.0                   # offset pushing selected experts out of argmin
GBIG = 50000.0               # -10000/TEMP gate logit offset for unselected

_CACHE = {}


def _build(bc=BC, n_cores=N_CORES, dbg=False):
    import concourse.bass as bass
    import concourse.tile as tile
    import concourse.mybir as mybir
    from concourse import bacc

    dt = mybir.dt
    f32, bf16, f16 = dt.float32, dt.bfloat16, dt.float16
    AF = mybir.ActivationFunctionType
    ALU = mybir.AluOpType
    AX = mybir.AxisListType

    nt = bc // P
    nch = max(1, nt // CHUNK_T)
    cht = nt // nch

    nc = bacc.Bacc("TRN2", target_bir_lowering=False, debug=False,
                   num_devices=n_cores)

    # ---- DRAM parameters (host pre-arranged layouts) ----
    zth = nc.dram_tensor("zth", [D, bc], bf16, kind="ExternalInput").ap()
    ztl = nc.dram_tensor("ztl", [D, bc], bf16, kind="ExternalInput").ap()
    nf = nc.dram_tensor("nf", [nt, P, 1], f32, kind="ExternalInput").ap()
    w1h = nc.dram_tensor("w1h", [K, D, D], bf16, kind="ExternalInput").ap()
    w1l = nc.dram_tensor("w1l", [K, D, D], bf16, kind="ExternalInput").ap()
    w2h = nc.dram_tensor("w2h", [K, D, C], bf16, kind="ExternalInput").ap()
    w2l = nc.dram_tensor("w2l", [K, D, C], bf16, kind="ExternalInput").ap()
    b1r = nc.dram_tensor("b1r", [P, K * MD], f32, kind="ExternalInput").ap()
    b2h = nc.dram_tensor("b2h", [1, K * C], bf16, kind="ExternalInput").ap()
    b2l = nc.dram_tensor("b2l", [1, K * C], bf16, kind="ExternalInput").ap()
    out = nc.dram_tensor("out", [bc, C], f32, kind="ExternalOutput").ap()
    gout = nc.dram_tensor("gout", [bc, K], f32, kind="ExternalOutput").ap()
    if dbg:
        nt_ = bc // P
        Ld = nc.dram_tensor("Ld", [nt_, P, K * C], f32,
                            kind="ExternalOutput").ap()
        confd = nc.dram_tensor("confd", [nt_, P, K], f32,
                               kind="ExternalOutput").ap()
        S2d = nc.dram_tensor("S2d", [nt_, P, K * K], f32,
                             kind="ExternalOutput").ap()
        seld = nc.dram_tensor("seld", [nt_, P, K], f32,
                              kind="ExternalOutput").ap()
        Td = nc.dram_tensor("Td", [nt_, P, K * K], f32,
                            kind="ExternalOutput").ap()
        dgd = nc.dram_tensor("dgd", [nt_, P, K], f32,
                             kind="ExternalOutput").ap()

    zth_v = zth.rearrange("(q p) s -> p q s", p=P)
    ztl_v = ztl.rearrange("(q p) s -> p q s", p=P)
    out_v = out.rearrange("(t p) c -> t p c", p=P)
    gout_v = gout.rearrange("(t p) k -> t p k", p=P)

    with tile.TileContext(nc) as tc, ExitStack() as ctx:
        pools = {}
        for name, bufs in [("const", 1), ("zt", 2), ("w1", 2), ("w2", 2),
                           ("h", 2), ("L", 1), ("E", 1), ("escr", 2),
                           ("pscr", 2), ("sm", 2), ("acc", 2), ("b2", 2)]:
            pools[name] = ctx.enter_context(tc.tile_pool(name=name, bufs=bufs))
        pph = ctx.enter_context(tc.tile_pool(name="pph", bufs=2, space="PSUM"))
        ppl = ctx.enter_context(tc.tile_pool(name="ppl", bufs=2, space="PSUM"))

        # ---- constants ----
        b1s = pools["const"].tile([P, K * MD], f32, tag="b1s")
        nc.sync.dma_start(b1s[:], b1r[:])
        ones = pools["const"].tile([1, P], bf16, tag="ones")
        nc.gpsimd.memset(ones[:], 1.0)
        zeros = pools["const"].tile([P, cht * P], f32, tag="zeros")
        nc.gpsimd.memset(zeros[:], 0.0)

        for ch in range(nch):
            s0 = ch * cht * P  # first sample of chunk within core
            zh_t = pools["zt"].tile([P, QD, cht * P], bf16, tag="zh")
            nc.sync.dma_start(zh_t[:], zth_v[:, :, s0:s0 + cht * P])
            zl_t = pools["zt"].tile([P, QD, cht * P], bf16, tag="zl")
            nc.sync.dma_start(zl_t[:], ztl_v[:, :, s0:s0 + cht * P])

            L_t = [pools["L"].tile([P, K * C], f32, tag=f"L{t}", name=f"L{t}")
                   for t in range(cht)]
            E_t = [pools["E"].tile([P, K * C], f16, tag=f"E{t}", name=f"E{t}")
                   for t in range(cht)]
            Z_t = [pools["sm"].tile([P, K], f32, tag=f"Z{t}", name=f"Z{t}")
                   for t in range(cht)]
            mn_t = [pools["sm"].tile([P, K], f32, tag=f"mn{t}", name=f"mn{t}")
                    for t in range(cht)]

            for k in range(K):
                w1h_t = pools["w1"].tile([P, QD, D], bf16, tag="w1h")
                nc.sync.dma_start(
                    w1h_t[:], w1h[k].rearrange("(q p) d -> p q d", p=P))
                w1l_t = pools["w1"].tile([P, QD, D], bf16, tag="w1l")
                nc.sync.dma_start(
                    w1l_t[:], w1l[k].rearrange("(q p) d -> p q d", p=P))
                w2h_t = pools["w2"].tile([P, QD, C], bf16, tag="w2h")
                nc.sync.dma_start(
                    w2h_t[:], w2h[k].rearrange("(q p) c -> p q c", p=P))
                w2l_t = pools["w2"].tile([P, QD, C], bf16, tag="w2l")
                nc.sync.dma_start(
                    w2l_t[:], w2l[k].rearrange("(q p) c -> p q c", p=P))
                ksl = slice(k * C, (k + 1) * C)
                b2hs = pools["b2"].tile([1, C], bf16, tag="b2hs")
                nc.sync.dma_start(b2hs[:], b2h[:, ksl])
                b2ls = pools["b2"].tile([1, C], bf16, tag="b2ls")
                nc.sync.dma_start(b2ls[:], b2l[:, ksl])

                # ---- layer 1: hT[dout, s] for the whole chunk ----
                ph = pph.tile([P, MD, cht * P], f32, tag="ph")
                for m in range(MD):
                    n_mm = 0
                    for q in range(QD):
                        dsl = slice(m * P, (m + 1) * P)
                        for lh, rh in ((w1h_t, zh_t), (w1h_t, zl_t),
                                       (w1l_t, zh_t)):
                            nc.tensor.matmul(
                                ph[:, m, :], lhsT=lh[:, q, dsl],
                                rhs=rh[:, q, :],
                                start=(n_mm == 0), stop=(n_mm == 3 * QD - 1))
                            n_mm += 1

                h32 = pools["h"].tile([P, MD, cht * P], f32, tag="h32")
                for m in range(MD):
                    # h32 = relu(ph + b1) in fp32 (DVE: gpsimd can't read PSUM)
                    nc.vector.scalar_tensor_tensor(
                        out=h32[:, m, :], in0=ph[:, m, :],
                        scalar=b1s[:, k * MD + m: k * MD + m + 1],
                        in1=zeros[:], op0=ALU.add, op1=ALU.max)
                hh = pools["h"].tile([P, MD, cht * P], bf16, tag="hh")
                nc.scalar.activation(hh[:], h32[:], AF.Copy)
                hl = pools["h"].tile([P, MD, cht * P], bf16, tag="hl")
                nc.gpsimd.tensor_tensor(hl[:], h32[:], hh[:], op=ALU.subtract)

                # ---- layer 2 + bias + softmax stats, per tile ----
                for t in range(cht):
                    ssl = slice(t * P, (t + 1) * P)
                    pl = ppl.tile([P, 1024], f32, tag="pl")
                    for c0, c1 in ((0, 512), (512, C)):
                        csl = slice(c0, c1)
                        n_mm = 0
                        for q in range(QD):
                            for lh, rh in ((hh, w2h_t), (hh, w2l_t),
                                           (hl, w2h_t)):
                                nc.tensor.matmul(
                                    pl[:, csl], lhsT=lh[:, q, ssl],
                                    rhs=rh[:, q, csl], start=(n_mm == 0),
                                    stop=False)
                                n_mm += 1
                        nc.tensor.matmul(pl[:, csl], lhsT=ones[:],
                                         rhs=b2hs[:, csl], start=False,
                                         stop=False)
                        nc.tensor.matmul(pl[:, csl], lhsT=ones[:],
                                         rhs=b2ls[:, csl], start=False,
                                         stop=True)
                    ksl = slice(k * C, (k + 1) * C)

                    nc.scalar.activation(L_t[t][:, ksl], pl[:, :C], AF.Copy)
                    nc.vector.tensor_reduce(
                        mn_t[t][:, k:k + 1], L_t[t][:, ksl], axis=AX.X,
                        op=ALU.max, negate=True)
                    e32 = pools["escr"].tile([P, C], f32, tag="e32")
                    nc.scalar.activation(
                        e32[:], L_t[t][:, ksl], AF.Exp,
                        bias=mn_t[t][:, k:k + 1],
                        accum_out=Z_t[t][:, k:k + 1])
                    nc.vector.tensor_copy(E_t[t][:, ksl], e32[:])

            # ---- stage B: selection + gates + output, per tile ----
            for t in range(cht):
                tg = ch * cht + t
                sm = pools["sm"]
                T = sm.tile([P, K * K], f32, tag="T")
                nc.gpsimd.memset(T[:], 0.0)
                dg = sm.tile([P, K], f32, tag="dg")
                for i in range(K):
                    for j in range(i, K):
                        accum = dg[:, i:i + 1] if i == j \
                            else T[:, i * K + j: i * K + j + 1]
                        p16 = pools["pscr"].tile([P, C], f16, tag="p16",
                                                 name="p16")
                        nc.vector.scalar_tensor_tensor(
                            out=p16[:], in0=E_t[t][:, i * C:(i + 1) * C],
                            scalar=1.0, in1=E_t[t][:, j * C:(j + 1) * C],
                            op0=ALU.bypass, op1=ALU.mult, accum_out=accum)

                Sf = sm.tile([P, K * K], f32, tag="Sf")
                nc.vector.tensor_tensor(
                    Sf[:].rearrange("p (i j) -> p i j", i=K),
                    T[:].rearrange("p (i j) -> p i j", i=K),
                    T[:].rearrange("p (i j) -> p j i", i=K),
                    op=ALU.max)
                rq = sm.tile([P, K], f32, tag="rq")
                nc.vector.reciprocal(rq[:], dg[:])
                invn = sm.tile([P, K], f32, tag="invn")
                nc.scalar.activation(invn[:], rq[:], AF.Sqrt)
                S2 = sm.tile([P, K * K], f32, tag="S2")
                Sf3 = Sf[:].rearrange("p (i j) -> p i j", i=K)
                nc.vector.tensor_tensor(
                    S2[:].rearrange("p (i j) -> p i j", i=K), Sf3,
                    invn[:].unsqueeze(2).broadcast_to([P, K, K]), op=ALU.mult)
                nc.vector.tensor_tensor(
                    S2[:].rearrange("p (i j) -> p i j", i=K),
                    S2[:].rearrange("p (i j) -> p i j", i=K),
                    invn[:].unsqueeze(1).broadcast_to([P, K, K]), op=ALU.mult)

                conf = sm.tile([P, K], f32, tag="conf")
                nc.vector.reciprocal(conf[:], Z_t[t][:])
                nft = sm.tile([P, 1], f32, tag="nft")
                nc.sync.dma_start(nft[:], nf[tg])

                cm = sm.tile([P, 1], f32, tag="cm")
                nc.vector.tensor_reduce(cm[:], conf[:], axis=AX.X, op=ALU.max)
                sel = sm.tile([P, K], f32, tag="sel")
                nc.vector.tensor_scalar(
                    out=sel[:], in0=conf[:], scalar1=cm[:], scalar2=None,
                    op0=ALU.is_equal)
                msp = sm.tile([P, K * K], f32, tag="msp")
                msp3 = msp[:].rearrange("p (i j) -> p i j", i=K)
                S23 = S2[:].rearrange("p (i j) -> p i j", i=K)
                nc.vector.tensor_tensor(
                    msp3, S23, sel[:].unsqueeze(1).broadcast_to([P, K, K]),
                    op=ALU.mult)
                msim = sm.tile([P, K], f32, tag="msim")
                nc.vector.tensor_reduce(msim[:], msp3, axis=AX.X, op=ALU.add)

                score = sm.tile([P, K], f32, tag="score")
                smin = sm.tile([P, 1], f32, tag="smin")
                oh = sm.tile([P, K], f32, tag="oh")
                ga = sm.tile([P, 1], f32, tag="ga")
                ohg = sm.tile([P, K], f32, tag="ohg")
                scol = sm.tile([P, K], f32, tag="scol")
                for tr in range(1, K - 1):
                    nc.vector.scalar_tensor_tensor(
                        out=score[:], in0=sel[:], scalar=BIG, in1=msim[:],
                        op0=ALU.mult, op1=ALU.add)
                    nc.vector.tensor_reduce(smin[:], score[:], axis=AX.X,
                                            op=ALU.min)
                    nc.vector.tensor_scalar(
                        out=oh[:], in0=score[:], scalar1=smin[:], scalar2=None,
                        op0=ALU.is_equal)
                    nc.vector.tensor_scalar(
                        out=ga[:], in0=nft[:], scalar1=float(tr), scalar2=None,
                        op0=ALU.is_gt)  # ga = (n > tr)
                    nc.vector.tensor_scalar(
                        out=ohg[:], in0=oh[:], scalar1=ga[:], scalar2=None,
                        op0=ALU.mult)
                    nc.vector.tensor_tensor(sel[:], sel[:], ohg[:], op=ALU.add)
                    nc.vector.tensor_tensor(
                        msp3, S23, ohg[:].unsqueeze(1).broadcast_to([P, K, K]),
                        op=ALU.mult)
                    nc.vector.tensor_reduce(scol[:], msp3, axis=AX.X,
                                            op=ALU.add)
                    nc.vector.tensor_tensor(msim[:], msim[:], scol[:],
                                            op=ALU.max)

                if dbg:
                    nc.sync.dma_start(Ld[tg], L_t[t][:])
                    nc.sync.dma_start(confd[tg], conf[:])
                    nc.sync.dma_start(S2d[tg], S2[:])
                    nc.sync.dma_start(seld[tg], sel[:])
                    nc.sync.dma_start(Td[tg], Sf[:])
                    nc.sync.dma_start(dgd[tg], dg[:])

                u0 = sm.tile([P, K], f32, tag="u0")
                nc.vector.tensor_scalar(
                    out=u0[:], in0=conf[:], scalar1=5.0, scalar2=None,
                    op0=ALU.mult)
                u = sm.tile([P, K], f32, tag="u")
                nc.vector.scalar_tensor_tensor(
                    out=u[:], in0=sel[:], scalar=GBIG, in1=u0[:],
                    op0=ALU.mult, op1=ALU.add)
                gm = sm.tile([P, 1], f32, tag="gm")
                nc.vector.tensor_reduce(gm[:], u[:], axis=AX.X, op=ALU.max,
                                        negate=True)
                eg = sm.tile([P, K], f32, tag="eg")
                zg = sm.tile([P, 1], f32, tag="zg")
                nc.scalar.activation(eg[:], u[:], AF.Exp, bias=gm[:],
                                     accum_out=zg[:])
                rg = sm.tile([P, 1], f32, tag="rg")
                nc.vector.reciprocal(rg[:], zg[:])
                gt = sm.tile([P, K], f32, tag="gt")
                nc.vector.tensor_scalar(
                    out=gt[:], in0=eg[:], scalar1=rg[:], scalar2=None,
                    op0=ALU.mult)
                nc.sync.dma_start(gout_v[tg], gt[:])

                acc = pools["acc"].tile([P, C], f32, tag="acc")
                nc.vector.tensor_scalar(
                    out=acc[:], in0=L_t[t][:, 0:C], scalar1=gt[:, 0:1],
                    scalar2=None, op0=ALU.mult)
                for k in range(1, K):
                    nc.vector.scalar_tensor_tensor(
                        out=acc[:], in0=L_t[t][:, k * C:(k + 1) * C],
                        scalar=gt[:, k:k + 1], in1=acc[:],
                        op0=ALU.mult, op1=ALU.add)
                nc.sync.dma_start(out_v[tg], acc[:])

    nc.compile()
    return nc


def _split_bf16(x):
    hi = x.astype(ml_dtypes.bfloat16)
    lo = (x - hi.astype(np.float32)).astype(ml_dtypes.bfloat16)
    return hi, lo


def kernel(z, n_exp, W1, b1, W2, b2):
    from concourse import bass_utils

    key = (BC, N_CORES)
    if key not in _CACHE:
        _CACHE[key] = _build()
    nc = _CACHE[key]

    z = np.asarray(z, dtype=np.float32)
    n_exp = np.asarray(n_exp)
    W1 = np.asarray(W1, dtype=np.float32)
    b1 = np.asarray(b1, dtype=np.float32)
    W2 = np.asarray(W2, dtype=np.float32)
    b2 = np.asarray(b2, dtype=np.float32)

    w1h, w1l = _split_bf16(W1)
    w2h, w2l = _split_bf16(W2)
    b2h, b2l = _split_bf16(b2.reshape(1, K * C))
    # b1 rearranged so b1r[p, k*MD+m] = b1[k, m*128+p]
    b1r = np.ascontiguousarray(
        b1.reshape(K, MD, P).transpose(2, 0, 1).reshape(P, K * MD))
    zT = np.ascontiguousarray(z.T)  # [D, B]
    zh, zl = _split_bf16(zT)
    nf = n_exp.astype(np.float32)

    in_maps = []
    for c in range(N_CORES):
        ssl = slice(c * BC, (c + 1) * BC)
        in_maps.append({
            "zth": np.ascontiguousarray(zh[:, ssl]),
            "ztl": np.ascontiguousarray(zl[:, ssl]),
            "nf": np.ascontiguousarray(nf[ssl].reshape(NT, P, 1)),
            "w1h": w1h, "w1l": w1l, "w2h": w2h, "w2l": w2l,
            "b1r": b1r, "b2h": b2h, "b2l": b2l,
        })

    trace = bool(int(os.environ.get("BK_TRACE", "0")))
    res = bass_utils.run_bass_kernel_spmd(
        nc, in_maps, core_ids=list(range(N_CORES)), trace=trace)
    kernel.last_result = res

    logits = np.concatenate([res.results[c]["out"] for c in range(N_CORES)], 0)
    gates = np.concatenate([res.results[c]["gout"] for c in range(N_CORES)], 0)
    return logits, gates


# revision 31
# speedup vs baseline: 1.1578x; 1.1578x over previous
"""Trainium2 Bass kernel for the MoE-routing branch module.

Per sample: 8 expert MLPs (512->512 relu, 512->1000), softmax confidence
per expert, greedy diversity-based expert selection, temperature-softmax
gates over selected experts, gated combination of expert logits.

Sharding: data-parallel over batch B=8192 across 8 cores (1024 samples
each); expert weights replicated.

Matmul precision: fp32 operands are split hi/lo into bf16 on the host and
the products are computed with 3 bf16 matmul passes (hi*hi + hi*lo +
lo*hi), accumulating in fp32 PSUM. Residual error ~2^-18 per term keeps
the greedy selection bit-faithful to the fp32 reference (validated: the
selection tolerates logit perturbations up to ~1e-4; this scheme gives
~2e-6).
"""

import os
import numpy as np
import ml_dtypes
from contextlib import ExitStack

B, D, C, K = 8192, 512, 1000, 8
N_CORES = 8
P = 128
BC = B // N_CORES            # samples per core
NT = BC // P                 # sample tiles per core
CHUNK_T = 2                  # tiles per weight-streaming chunk
QD = D // P                  # contraction chunks (4)
MD = D // P                  # layer-1 output chunks (4)
BIG = 10.0                   # offset pushing selected experts out of argmin
GBIG = 50000.0               # -10000/TEMP gate logit offset for unselected

_CACHE = {}


def _build(bc=BC, n_cores=N_CORES, dbg=False):
    import concourse.bass as bass
    import concourse.tile as tile
    import concourse.mybir as mybir
    from concourse import bacc

    dt = mybir.dt
    f32, bf16, f16 = dt.float32, dt.bfloat16, dt.float16
    AF = mybir.ActivationFunctionType
    ALU = mybir.AluOpType
    AX = mybir.AxisListType

    nt = bc // P
    nch = max(1, nt // CHUNK_T)
    cht = nt // nch

    nc = bacc.Bacc("TRN2", target_bir_lowering=False, debug=False,
                   num_devices=n_cores)

    # ---- DRAM parameters (host pre-arranged layouts) ----
    zth = nc.dram_tensor("zth", [D, bc], bf16, kind="ExternalInput").ap()
    ztl = nc.dram_tensor("ztl", [D, bc], bf16, kind="ExternalInput").ap()
    nf = nc.dram_tensor("nf", [nt, P, 1], f32, kind="ExternalInput").ap()
    w1h = nc.dram_tensor("w1h", [K, D, D], bf16, kind="ExternalInput").ap()
    w1l = nc.dram_tensor("w1l", [K, D, D], bf16, kind="ExternalInput").ap()
    w2h = nc.dram_tensor("w2h", [K, D, C], bf16, kind="ExternalInput").ap()
    w2l = nc.dram_tensor("w2l", [K, D, C], bf16, kind="ExternalInput").ap()
    b1r = nc.dram_tensor("b1r", [P, K * MD], f32, kind="ExternalInput").ap()
    b2hl = nc.dram_tensor("b2hl", [2, K * C], bf16, kind="ExternalInput").ap()
    out = nc.dram_tensor("out", [bc, C], f32, kind="ExternalOutput").ap()
    gout = nc.dram_tensor("gout", [bc, K], f32, kind="ExternalOutput").ap()
    if dbg:
        nt_ = bc // P
        Ld = nc.dram_tensor("Ld", [nt_, P, K * C], f32,
                            kind="ExternalOutput").ap()
        confd = nc.dram_tensor("confd", [nt_, P, K], f32,
                               kind="ExternalOutput").ap()
        S2d = nc.dram_tensor("S2d", [nt_, P, K * K], f32,
                             kind="ExternalOutput").ap()
        seld = nc.dram_tensor("seld", [nt_, P, K], f32,
                              kind="ExternalOutput").ap()
        Td = nc.dram_tensor("Td", [nt_, P, K * K], f32,
                            kind="ExternalOutput").ap()
        dgd = nc.dram_tensor("dgd", [nt_, P, K], f32,
                             kind="ExternalOutput").ap()

    zth_v = zth.rearrange("(q p) s -> p q s", p=P)
    ztl_v = ztl.rearrange("(q p) s -> p q s", p=P)
    out_v = out.rearrange("(t p) c -> t p c", p=P)
    gout_v = gout.rearrange("(t p) k -> t p k", p=P)

    with tile.TileContext(nc) as tc, ExitStack() as ctx:
        pools = {}
        for name, bufs in [("const", 1), ("zt", 2), ("w1", 2), ("w2", 2),
                           ("h", 2), ("L", 1), ("E", 1),
                           ("pscr", 2), ("sm", 2), ("acc", 2), ("b2", 2)]:
            pools[name] = ctx.enter_context(tc.tile_pool(name=name, bufs=bufs))
        pph = ctx.enter_context(tc.tile_pool(name="pph", bufs=2, space="PSUM"))
        ppl = ctx.enter_context(tc.tile_pool(name="ppl", bufs=2, space="PSUM"))

        # ---- constants ----
        b1s = pools["const"].tile([P, K * MD], f32, tag="b1s")
        nc.sync.dma_start(b1s[:], b1r[:])
        ones2 = pools["const"].tile([2, P], bf16, tag="ones2")
        nc.gpsimd.memset(ones2[:], 1.0)
        zeros = pools["const"].tile([P, cht * P], f32, tag="zeros")
        nc.gpsimd.memset(zeros[:], 0.0)
        nfall = pools["const"].tile([P, nt], f32, tag="nfall")
        nc.sync.dma_start(nfall[:], nf.rearrange("t p o -> p (t o)"))

        for ch in range(nch):
            s0 = ch * cht * P  # first sample of chunk within core
            zh_t = pools["zt"].tile([P, QD, cht * P], bf16, tag="zh")
            nc.sync.dma_start(zh_t[:], zth_v[:, :, s0:s0 + cht * P])
            zl_t = pools["zt"].tile([P, QD, cht * P], bf16, tag="zl")
            nc.sync.dma_start(zl_t[:], ztl_v[:, :, s0:s0 + cht * P])

            L_t = [pools["L"].tile([P, K * C], f32, tag=f"L{t}", name=f"L{t}")
                   for t in range(cht)]
            E_t = [pools["E"].tile([P, K * C], f16, tag=f"E{t}", name=f"E{t}")
                   for t in range(cht)]
            Z_t = [pools["sm"].tile([P, K], f32, tag=f"Z{t}", name=f"Z{t}")
                   for t in range(cht)]
            mn_t = [pools["sm"].tile([P, K], f32, tag=f"mn{t}", name=f"mn{t}")
                    for t in range(cht)]
            T_t = [pools["sm"].tile([P, K * K], f32, tag=f"T{t}", name=f"T{t}")
                   for t in range(cht)]
            dg_t = [pools["sm"].tile([P, K], f32, tag=f"dg{t}", name=f"dg{t}")
                    for t in range(cht)]
            for t in range(cht):
                nc.gpsimd.memset(T_t[t][:], 0.0)

            for k in range(K):
                w1h_t = pools["w1"].tile([P, QD, D], bf16, tag="w1h")
                nc.sync.dma_start(
                    w1h_t[:], w1h[k].rearrange("(q p) d -> p q d", p=P))
                w1l_t = pools["w1"].tile([P, QD, D], bf16, tag="w1l")
                nc.sync.dma_start(
                    w1l_t[:], w1l[k].rearrange("(q p) d -> p q d", p=P))
                w2h_t = pools["w2"].tile([P, QD, C], bf16, tag="w2h")
                nc.sync.dma_start(
                    w2h_t[:], w2h[k].rearrange("(q p) c -> p q c", p=P))
                w2l_t = pools["w2"].tile([P, QD, C], bf16, tag="w2l")
                nc.sync.dma_start(
                    w2l_t[:], w2l[k].rearrange("(q p) c -> p q c", p=P))
                ksl = slice(k * C, (k + 1) * C)
                b2s = pools["b2"].tile([2, C], bf16, tag="b2s")
                nc.sync.dma_start(b2s[:], b2hl[:, ksl])

                # ---- layer 1: hT[dout, s] for the whole chunk ----
                ph = pph.tile([P, MD, cht * P], f32, tag="ph")
                for m in range(MD):
                    n_mm = 0
                    for q in range(QD):
                        dsl = slice(m * P, (m + 1) * P)
                        for lh, rh in ((w1h_t, zh_t), (w1h_t, zl_t),
                                       (w1l_t, zh_t)):
                            nc.tensor.matmul(
                                ph[:, m, :], lhsT=lh[:, q, dsl],
                                rhs=rh[:, q, :],
                                start=(n_mm == 0), stop=(n_mm == 3 * QD - 1))
                            n_mm += 1

                h32 = pools["h"].tile([P, MD, cht * P], f32, tag="h32")
                for m in range(MD):
                    # h32 = relu(ph + b1) in fp32 (DVE: gpsimd can't read PSUM)
                    nc.vector.scalar_tensor_tensor(
                        out=h32[:, m, :], in0=ph[:, m, :],
                        scalar=b1s[:, k * MD + m: k * MD + m + 1],
                        in1=zeros[:], op0=ALU.add, op1=ALU.max)
                hh = pools["h"].tile([P, MD, cht * P], bf16, tag="hh")
                nc.scalar.activation(hh[:], h32[:], AF.Copy)
                hl = pools["h"].tile([P, MD, cht * P], bf16, tag="hl")
                nc.gpsimd.tensor_tensor(hl[:], h32[:], hh[:], op=ALU.subtract)

                # ---- layer 2 + bias + softmax stats, per tile ----
                for t in range(cht):
                    ssl = slice(t * P, (t + 1) * P)
                    pl = ppl.tile([P, 1024], f32, tag="pl")
                    for c0, c1 in ((0, 512), (512, C)):
                        csl = slice(c0, c1)
                        n_mm = 0
                        for q in range(QD):
                            for lh, rh in ((hh, w2h_t), (hh, w2l_t),
                                           (hl, w2h_t)):
                                nc.tensor.matmul(
                                    pl[:, csl], lhsT=lh[:, q, ssl],
                                    rhs=rh[:, q, csl], start=(n_mm == 0),
                                    stop=False)
                                n_mm += 1
                        nc.tensor.matmul(pl[:, csl], lhsT=ones2[:],
                                         rhs=b2s[:, csl], start=False,
                                         stop=True)
                    ksl = slice(k * C, (k + 1) * C)

                    nc.scalar.activation(L_t[t][:, ksl], pl[:, :C], AF.Copy)
                    nc.vector.tensor_reduce(
                        mn_t[t][:, k:k + 1], L_t[t][:, ksl], axis=AX.X,
                        op=ALU.max, negate=True)
                    # exp straight to fp16; accum taps pre-cast fp32 (probed)
                    nc.scalar.activation(
                        E_t[t][:, ksl], L_t[t][:, ksl], AF.Exp,
                        bias=mn_t[t][:, k:k + 1],
                        accum_out=Z_t[t][:, k:k + 1])
                    # pairwise E products with earlier experts, spread across
                    # engines; diagonal via ACT Square
                    pa16 = pools["pscr"].tile([P, C], f16, tag="pa16",
                                              name="pa16")
                    nc.scalar.activation(pa16[:], E_t[t][:, ksl], AF.Square,
                                         accum_out=dg_t[t][:, k:k + 1])
                    for i in range(k):
                        n_off = i * K + k
                        tcell = T_t[t][:, n_off:n_off + 1]
                        if n_off % 7 in (1, 3, 5):
                            # gpsimd can't run STT: mult there, reduce on DVE
                            pg = pools["pscr"].tile([P, C], f16, tag="pg16",
                                                    name="pg16")
                            nc.gpsimd.tensor_tensor(
                                pg[:], E_t[t][:, i * C:(i + 1) * C],
                                E_t[t][:, ksl], op=ALU.mult)
                            nc.vector.tensor_reduce(tcell, pg[:], axis=AX.X,
                                                    op=ALU.add)
                        else:
                            p16 = pools["pscr"].tile([P, C], f16, tag="p16",
                                                     name="p16")
                            nc.vector.scalar_tensor_tensor(
                                out=p16[:], in0=E_t[t][:, i * C:(i + 1) * C],
                                scalar=1.0, in1=E_t[t][:, ksl],
                                op0=ALU.bypass, op1=ALU.mult,
                                accum_out=tcell)

            # ---- stage B: selection + gates + output, per tile ----
            for t in range(cht):
                tg = ch * cht + t
                sm = pools["sm"]
                T = T_t[t]
                dg = dg_t[t]
                Sf = sm.tile([P, K * K], f32, tag="Sf")
                nc.vector.tensor_tensor(
                    Sf[:].rearrange("p (i j) -> p i j", i=K),
                    T[:].rearrange("p (i j) -> p i j", i=K),
                    T[:].rearrange("p (i j) -> p j i", i=K),
                    op=ALU.max)
                rq = sm.tile([P, K], f32, tag="rq")
                nc.vector.reciprocal(rq[:], dg[:])
                invn = sm.tile([P, K], f32, tag="invn")
                nc.scalar.activation(invn[:], rq[:], AF.Sqrt)
                S2 = sm.tile([P, K * K], f32, tag="S2")
                Sf3 = Sf[:].rearrange("p (i j) -> p i j", i=K)
                nc.vector.tensor_tensor(
                    S2[:].rearrange("p (i j) -> p i j", i=K), Sf3,
                    invn[:].unsqueeze(2).broadcast_to([P, K, K]), op=ALU.mult)
                nc.vector.tensor_tensor(
                    S2[:].rearrange("p (i j) -> p i j", i=K),
                    S2[:].rearrange("p (i j) -> p i j", i=K),
                    invn[:].unsqueeze(1).broadcast_to([P, K, K]), op=ALU.mult)

                conf = sm.tile([P, K], f32, tag="conf")
                nc.vector.reciprocal(conf[:], Z_t[t][:])
                nft = nfall[:, tg:tg + 1]

                cm = sm.tile([P, 1], f32, tag="cm")
                nc.vector.tensor_reduce(cm[:], conf[:], axis=AX.X, op=ALU.max)
                sel = sm.tile([P, K], f32, tag="sel")
                nc.vector.tensor_scalar(
                    out=sel[:], in0=conf[:], scalar1=cm[:], scalar2=None,
                    op0=ALU.is_equal)
                msp = sm.tile([P, K * K], f32, tag="msp")
                msp3 = msp[:].rearrange("p (i j) -> p i j", i=K)
                S23 = S2[:].rearrange("p (i j) -> p i j", i=K)
                nc.vector.tensor_tensor(
                    msp3, S23, sel[:].unsqueeze(1).broadcast_to([P, K, K]),
                    op=ALU.mult)
                msim = sm.tile([P, K], f32, tag="msim")
                nc.vector.tensor_reduce(msim[:], msp3, axis=AX.X, op=ALU.add)

                score = sm.tile([P, K], f32, tag="score")
                smin = sm.tile([P, 1], f32, tag="smin")
                oh = sm.tile([P, K], f32, tag="oh")
                ga = sm.tile([P, 1], f32, tag="ga")
                ohg = sm.tile([P, K], f32, tag="ohg")
                scol = sm.tile([P, K], f32, tag="scol")
                for tr in range(1, K - 1):
                    nc.vector.scalar_tensor_tensor(
                        out=score[:], in0=sel[:], scalar=BIG, in1=msim[:],
                        op0=ALU.mult, op1=ALU.add)
                    nc.vector.tensor_reduce(smin[:], score[:], axis=AX.X,
                                            op=ALU.min)
                    nc.vector.tensor_scalar(
                        out=oh[:], in0=score[:], scalar1=smin[:], scalar2=None,
                        op0=ALU.is_equal)
                    nc.vector.tensor_scalar(
                        out=ga[:], in0=nft, scalar1=float(tr), scalar2=None,
                        op0=ALU.is_gt)  # ga = (n > tr)
                    nc.vector.tensor_scalar(
                        out=ohg[:], in0=oh[:], scalar1=ga[:], scalar2=None,
                        op0=ALU.mult)
                    nc.vector.tensor_tensor(sel[:], sel[:], ohg[:], op=ALU.add)
                    nc.vector.tensor_tensor(
                        msp3, S23, ohg[:].unsqueeze(1).broadcast_to([P, K, K]),
                        op=ALU.mult)
                    nc.vector.tensor_reduce(scol[:], msp3, axis=AX.X,
                                            op=ALU.add)
                    nc.vector.tensor_tensor(msim[:], msim[:], scol[:],
                                            op=ALU.max)

                if dbg:
                    nc.sync.dma_start(Ld[tg], L_t[t][:])
                    nc.sync.dma_start(confd[tg], conf[:])
                    nc.sync.dma_start(S2d[tg], S2[:])
                    nc.sync.dma_start(seld[tg], sel[:])
                    nc.sync.dma_start(Td[tg], Sf[:])
                    nc.sync.dma_start(dgd[tg], dg[:])

                u0 = sm.tile([P, K], f32, tag="u0")
                nc.vector.tensor_scalar(
                    out=u0[:], in0=conf[:], scalar1=5.0, scalar2=None,
                    op0=ALU.mult)
                u = sm.tile([P, K], f32, tag="u")
                nc.vector.scalar_tensor_tensor(
                    out=u[:], in0=sel[:], scalar=GBIG, in1=u0[:],
                    op0=ALU.mult, op1=ALU.add)
                gm = sm.tile([P, 1], f32, tag="gm")
                nc.vector.tensor_reduce(gm[:], u[:], axis=AX.X, op=ALU.max,
                                        negate=True)
                eg = sm.tile([P, K], f32, tag="eg")
                zg = sm.tile([P, 1], f32, tag="zg")
                nc.scalar.activation(eg[:], u[:], AF.Exp, bias=gm[:],
                                     accum_out=zg[:])
                rg = sm.tile([P, 1], f32, tag="rg")
                nc.vector.reciprocal(rg[:], zg[:])
                gt = sm.tile([P, K], f32, tag="gt")
                nc.vector.tensor_scalar(
                    out=gt[:], in0=eg[:], scalar1=rg[:], scalar2=None,
                    op0=ALU.mult)
                nc.scalar.dma_start(gout_v[tg], gt[:])

                acc = pools["acc"].tile([P, C], f32, tag="acc")
                nc.vector.tensor_scalar(
                    out=acc[:], in0=L_t[t][:, 0:C], scalar1=gt[:, 0:1],
                    scalar2=None, op0=ALU.mult)
                for k in range(1, K):
                    nc.vector.scalar_tensor_tensor(
                        out=acc[:], in0=L_t[t][:, k * C:(k + 1) * C],
                        scalar=gt[:, k:k + 1], in1=acc[:],
                        op0=ALU.mult, op1=ALU.add)
                nc.scalar.dma_start(out_v[tg], acc[:])

    nc.compile()
    return nc


def _split_bf16(x):
    hi = x.astype(ml_dtypes.bfloat16)
    lo = (x - hi.astype(np.float32)).astype(ml_dtypes.bfloat16)
    return hi, lo


def kernel(z, n_exp, W1, b1, W2, b2):
    from concourse import bass_utils

    key = (BC, N_CORES)
    if key not in _CACHE:
        _CACHE[key] = _build()
    nc = _CACHE[key]

    z = np.asarray(z, dtype=np.float32)
    n_exp = np.asarray(n_exp)
    W1 = np.asarray(W1, dtype=np.float32)
    b1 = np.asarray(b1, dtype=np.float32)
    W2 = np.asarray(W2, dtype=np.float32)
    b2 = np.asarray(b2, dtype=np.float32)

    w1h, w1l = _split_bf16(W1)
    w2h, w2l = _split_bf16(W2)
    b2h, b2l = _split_bf16(b2.reshape(1, K * C))
    b2hl = np.ascontiguousarray(np.concatenate([b2h, b2l], axis=0))
    # b1 rearranged so b1r[p, k*MD+m] = b1[k, m*128+p]
    b1r = np.ascontiguousarray(
        b1.reshape(K, MD, P).transpose(2, 0, 1).reshape(P, K * MD))
    zT = np.ascontiguousarray(z.T)  # [D, B]
    zh, zl = _split_bf16(zT)
    nf = n_exp.astype(np.float32)

    in_maps = []
    for c in range(N_CORES):
        ssl = slice(c * BC, (c + 1) * BC)
        in_maps.append({
            "zth": np.ascontiguousarray(zh[:, ssl]),
            "ztl": np.ascontiguousarray(zl[:, ssl]),
            "nf": np.ascontiguousarray(nf[ssl].reshape(NT, P, 1)),
            "w1h": w1h, "w1l": w1l, "w2h": w2h, "w2l": w2l,
            "b1r": b1r, "b2hl": b2hl,
        })

    trace = bool(int(os.environ.get("BK_TRACE", "0")))
    res = bass_utils.run_bass_kernel_spmd(
        nc, in_maps, core_ids=list(range(N_CORES)), trace=trace)
    kernel.last_result = res

    logits = np.concatenate([res.results[c]["out"] for c in range(N_CORES)], 0)
    gates = np.concatenate([res.results[c]["gout"] for c in range(N_CORES)], 0)
    return logits, gates


# revision 35
# speedup vs baseline: 1.3265x; 1.1457x over previous
"""Trainium2 Bass kernel for the MoE-routing branch module.

Per sample: 8 expert MLPs (512->512 relu, 512->1000), softmax confidence
per expert, greedy diversity-based expert selection, temperature-softmax
gates over selected experts, gated combination of expert logits.

Sharding: data-parallel over batch B=8192 across 8 cores (1024 samples
each); expert weights replicated.

Matmul precision: fp32 operands are split hi/lo into bf16 on the host and
the products are computed with 3 bf16 matmul passes (hi*hi + hi*lo +
lo*hi), accumulating in fp32 PSUM. Residual error ~2^-18 per term keeps
the greedy selection bit-faithful to the fp32 reference (validated: the
selection tolerates logit perturbations up to ~1e-4; this scheme gives
~2e-6).
"""

import os
import numpy as np
import ml_dtypes
from contextlib import ExitStack

B, D, C, K = 8192, 512, 1000, 8
N_CORES = 8
P = 128
BC = B // N_CORES            # samples per core
NT = BC // P                 # sample tiles per core
CHUNK_T = 2                  # tiles per weight-streaming chunk
QD = D // P                  # contraction chunks (4)
MD = D // P                  # layer-1 output chunks (4)
BIG = 10.0                   # offset pushing selected experts out of argmin
GBIG = 50000.0               # -10000/TEMP gate logit offset for unselected

_CACHE = {}


def _build(bc=BC, n_cores=N_CORES, dbg=False):
    import concourse.bass as bass
    import concourse.tile as tile
    import concourse.mybir as mybir
    from concourse import bacc

    dt = mybir.dt
    f32, bf16, f16 = dt.float32, dt.bfloat16, dt.float16
    AF = mybir.ActivationFunctionType
    ALU = mybir.AluOpType
    AX = mybir.AxisListType

    nt = bc // P
    nch = max(1, nt // CHUNK_T)
    cht = nt // nch

    nc = bacc.Bacc("TRN2", target_bir_lowering=False, debug=False,
                   num_devices=n_cores)

    # ---- DRAM parameters (host pre-arranged layouts) ----
    zth = nc.dram_tensor("zth", [D, bc], bf16, kind="ExternalInput").ap()
    ztl = nc.dram_tensor("ztl", [D, bc], bf16, kind="ExternalInput").ap()
    nf = nc.dram_tensor("nf", [nt, P, 1], f32, kind="ExternalInput").ap()
    w1h = nc.dram_tensor("w1h", [K, D, D], bf16, kind="ExternalInput").ap()
    w1l = nc.dram_tensor("w1l", [K, D, D], bf16, kind="ExternalInput").ap()
    w2h = nc.dram_tensor("w2h", [K, D, C], bf16, kind="ExternalInput").ap()
    w2l = nc.dram_tensor("w2l", [K, D, C], bf16, kind="ExternalInput").ap()
    b1r = nc.dram_tensor("b1r", [P, K * MD], f32, kind="ExternalInput").ap()
    b2hl = nc.dram_tensor("b2hl", [2, K * C], bf16, kind="ExternalInput").ap()
    out = nc.dram_tensor("out", [bc, C], f32, kind="ExternalOutput").ap()
    gout = nc.dram_tensor("gout", [bc, K], f32, kind="ExternalOutput").ap()
    if dbg:
        nt_ = bc // P
        Ld = nc.dram_tensor("Ld", [nt_, P, K * C], f32,
                            kind="ExternalOutput").ap()
        confd = nc.dram_tensor("confd", [nt_, P, K], f32,
                               kind="ExternalOutput").ap()
        S2d = nc.dram_tensor("S2d", [nt_, P, K * K], f32,
                             kind="ExternalOutput").ap()
        seld = nc.dram_tensor("seld", [nt_, P, K], f32,
                              kind="ExternalOutput").ap()
        Td = nc.dram_tensor("Td", [nt_, P, K * K], f32,
                            kind="ExternalOutput").ap()
        dgd = nc.dram_tensor("dgd", [nt_, P, K], f32,
                             kind="ExternalOutput").ap()

    zth_v = zth.rearrange("(q p) s -> p q s", p=P)
    ztl_v = ztl.rearrange("(q p) s -> p q s", p=P)
    out_v = out.rearrange("(t p) c -> t p c", p=P)
    gout_v = gout.rearrange("(t p) k -> t p k", p=P)

    with tile.TileContext(nc) as tc, ExitStack() as ctx:
        pools = {}
        for name, bufs in [("const", 1), ("zt", 2), ("w1", 2), ("w2", 2),
                           ("h", 2), ("L", 1), ("E", 1),
                           ("pscr", 2), ("sm", 2), ("acc", 2), ("b2", 2)]:
            pools[name] = ctx.enter_context(tc.tile_pool(name=name, bufs=bufs))
        pph = ctx.enter_context(tc.tile_pool(name="pph", bufs=2, space="PSUM"))
        ppl = ctx.enter_context(tc.tile_pool(name="ppl", bufs=2, space="PSUM"))

        # ---- constants ----
        b1s = pools["const"].tile([P, K * MD], f32, tag="b1s")
        nc.sync.dma_start(b1s[:], b1r[:])
        ones2 = pools["const"].tile([2, P], bf16, tag="ones2")
        nc.gpsimd.memset(ones2[:], 1.0)
        zeros = pools["const"].tile([P, cht * P], f32, tag="zeros")
        nc.gpsimd.memset(zeros[:], 0.0)
        nfall = pools["const"].tile([P, nt], f32, tag="nfall")
        nc.sync.dma_start(nfall[:], nf.rearrange("t p o -> p (t o)"))

        for ch in range(nch):
            s0 = ch * cht * P  # first sample of chunk within core
            zh_t = pools["zt"].tile([P, QD, cht * P], bf16, tag="zh")
            nc.gpsimd.dma_start(zh_t[:], zth_v[:, :, s0:s0 + cht * P])
            zl_t = pools["zt"].tile([P, QD, cht * P], bf16, tag="zl")
            nc.gpsimd.dma_start(zl_t[:], ztl_v[:, :, s0:s0 + cht * P])

            L_t = [pools["L"].tile([P, K * C], f32, tag=f"L{t}", name=f"L{t}")
                   for t in range(cht)]
            E_t = [pools["E"].tile([P, K * C], f16, tag=f"E{t}", name=f"E{t}")
                   for t in range(cht)]
            Z_t = [pools["sm"].tile([P, K], f32, tag=f"Z{t}", name=f"Z{t}")
                   for t in range(cht)]
            mn_t = [pools["sm"].tile([P, K], f32, tag=f"mn{t}", name=f"mn{t}")
                    for t in range(cht)]
            T_t = [pools["sm"].tile([P, K * K], f32, tag=f"T{t}", name=f"T{t}")
                   for t in range(cht)]
            dg_t = [pools["sm"].tile([P, K], f32, tag=f"dg{t}", name=f"dg{t}")
                    for t in range(cht)]
            for t in range(cht):
                nc.gpsimd.memset(T_t[t][:], 0.0)

            for k in range(K):
                w1h_t = pools["w1"].tile([P, QD, D], bf16, tag="w1h")
                nc.gpsimd.dma_start(
                    w1h_t[:], w1h[k].rearrange("(q p) d -> p q d", p=P))
                w1l_t = pools["w1"].tile([P, QD, D], bf16, tag="w1l")
                nc.gpsimd.dma_start(
                    w1l_t[:], w1l[k].rearrange("(q p) d -> p q d", p=P))
                w2h_t = pools["w2"].tile([P, QD, C], bf16, tag="w2h")
                nc.sync.dma_start(
                    w2h_t[:], w2h[k].rearrange("(q p) c -> p q c", p=P))
                w2l_t = pools["w2"].tile([P, QD, C], bf16, tag="w2l")
                nc.sync.dma_start(
                    w2l_t[:], w2l[k].rearrange("(q p) c -> p q c", p=P))
                ksl = slice(k * C, (k + 1) * C)
                b2s = pools["b2"].tile([2, C], bf16, tag="b2s")
                nc.gpsimd.dma_start(b2s[:], b2hl[:, ksl])

                # ---- layer 1: hT[dout, s] for the whole chunk ----
                ph = pph.tile([P, MD, cht * P], f32, tag="ph")
                for m in range(MD):
                    n_mm = 0
                    for q in range(QD):
                        dsl = slice(m * P, (m + 1) * P)
                        for lh, rh in ((w1h_t, zh_t), (w1h_t, zl_t),
                                       (w1l_t, zh_t)):
                            nc.tensor.matmul(
                                ph[:, m, :], lhsT=lh[:, q, dsl],
                                rhs=rh[:, q, :],
                                start=(n_mm == 0), stop=(n_mm == 3 * QD - 1))
                            n_mm += 1

                h32 = pools["h"].tile([P, MD, cht * P], f32, tag="h32")
                for m in range(MD):
                    # h32 = relu(ph + b1) in fp32 (DVE: gpsimd can't read PSUM)
                    nc.vector.scalar_tensor_tensor(
                        out=h32[:, m, :], in0=ph[:, m, :],
                        scalar=b1s[:, k * MD + m: k * MD + m + 1],
                        in1=zeros[:], op0=ALU.add, op1=ALU.max)
                hh = pools["h"].tile([P, MD, cht * P], bf16, tag="hh")
                nc.scalar.activation(hh[:], h32[:], AF.Copy)
                hl = pools["h"].tile([P, MD, cht * P], bf16, tag="hl")
                nc.gpsimd.tensor_tensor(hl[:], h32[:], hh[:], op=ALU.subtract)

                # ---- layer 2 + bias + softmax stats, per tile ----
                for t in range(cht):
                    ssl = slice(t * P, (t + 1) * P)
                    pl = ppl.tile([P, 1024], f32, tag="pl")
                    for c0, c1 in ((0, 512), (512, C)):
                        csl = slice(c0, c1)
                        n_mm = 0
                        for q in range(QD):
                            for lh, rh in ((hh, w2h_t), (hh, w2l_t),
                                           (hl, w2h_t)):
                                nc.tensor.matmul(
                                    pl[:, csl], lhsT=lh[:, q, ssl],
                                    rhs=rh[:, q, csl], start=(n_mm == 0),
                                    stop=False)
                                n_mm += 1
                        nc.tensor.matmul(pl[:, csl], lhsT=ones2[:],
                                         rhs=b2s[:, csl], start=False,
                                         stop=True)
                    ksl = slice(k * C, (k + 1) * C)

                    nc.scalar.activation(L_t[t][:, ksl], pl[:, :C], AF.Copy)
                    nc.vector.tensor_reduce(
                        mn_t[t][:, k:k + 1], L_t[t][:, ksl], axis=AX.X,
                        op=ALU.max, negate=True)
                    # exp straight to fp16; accum taps pre-cast fp32 (probed)
                    nc.scalar.activation(
                        E_t[t][:, ksl], L_t[t][:, ksl], AF.Exp,
                        bias=mn_t[t][:, k:k + 1],
                        accum_out=Z_t[t][:, k:k + 1])
                    # pairwise E products with earlier experts, spread across
                    # engines; diagonal via ACT Square
                    pa16 = pools["pscr"].tile([P, C], f16, tag="pa16",
                                              name="pa16")
                    nc.scalar.activation(pa16[:], E_t[t][:, ksl], AF.Square,
                                         accum_out=dg_t[t][:, k:k + 1])
                    for i in range(k):
                        n_off = i * K + k
                        tcell = T_t[t][:, n_off:n_off + 1]
                        p16 = pools["pscr"].tile([P, C], f16, tag="p16",
                                                 name="p16")
                        nc.vector.scalar_tensor_tensor(
                            out=p16[:], in0=E_t[t][:, i * C:(i + 1) * C],
                            scalar=1.0, in1=E_t[t][:, ksl],
                            op0=ALU.bypass, op1=ALU.mult,
                            accum_out=tcell)

            # ---- stage B: selection + gates + output, per tile ----
            for t in range(cht):
                tg = ch * cht + t
                sm = pools["sm"]
                T = T_t[t]
                dg = dg_t[t]
                Sf = sm.tile([P, K * K], f32, tag="Sf")
                nc.vector.tensor_tensor(
                    Sf[:].rearrange("p (i j) -> p i j", i=K),
                    T[:].rearrange("p (i j) -> p i j", i=K),
                    T[:].rearrange("p (i j) -> p j i", i=K),
                    op=ALU.max)
                rq = sm.tile([P, K], f32, tag="rq")
                nc.vector.reciprocal(rq[:], dg[:])
                invn = sm.tile([P, K], f32, tag="invn")
                nc.scalar.activation(invn[:], rq[:], AF.Sqrt)
                S2 = sm.tile([P, K * K], f32, tag="S2")
                Sf3 = Sf[:].rearrange("p (i j) -> p i j", i=K)
                nc.vector.tensor_tensor(
                    S2[:].rearrange("p (i j) -> p i j", i=K), Sf3,
                    invn[:].unsqueeze(2).broadcast_to([P, K, K]), op=ALU.mult)
                nc.vector.tensor_tensor(
                    S2[:].rearrange("p (i j) -> p i j", i=K),
                    S2[:].rearrange("p (i j) -> p i j", i=K),
                    invn[:].unsqueeze(1).broadcast_to([P, K, K]), op=ALU.mult)

                conf = sm.tile([P, K], f32, tag="conf")
                nc.vector.reciprocal(conf[:], Z_t[t][:])
                nft = nfall[:, tg:tg + 1]

                cm = sm.tile([P, 1], f32, tag="cm")
                nc.vector.tensor_reduce(cm[:], conf[:], axis=AX.X, op=ALU.max)
                sel = sm.tile([P, K], f32, tag="sel")
                nc.vector.tensor_scalar(
                    out=sel[:], in0=conf[:], scalar1=cm[:], scalar2=None,
                    op0=ALU.is_equal)
                msp = sm.tile([P, K * K], f32, tag="msp")
                msp3 = msp[:].rearrange("p (i j) -> p i j", i=K)
                S23 = S2[:].rearrange("p (i j) -> p i j", i=K)
                nc.vector.tensor_tensor(
                    msp3, S23, sel[:].unsqueeze(1).broadcast_to([P, K, K]),
                    op=ALU.mult)
                msim = sm.tile([P, K], f32, tag="msim")
                nc.vector.tensor_reduce(msim[:], msp3, axis=AX.X, op=ALU.add)

                score = sm.tile([P, K], f32, tag="score")
                smin = sm.tile([P, 1], f32, tag="smin")
                oh = sm.tile([P, K], f32, tag="oh")
                ga = sm.tile([P, 1], f32, tag="ga")
                ohg = sm.tile([P, K], f32, tag="ohg")
                scol = sm.tile([P, K], f32, tag="scol")
                for tr in range(1, K - 1):
                    nc.vector.scalar_tensor_tensor(
                        out=score[:], in0=sel[:], scalar=BIG, in1=msim[:],
                        op0=ALU.mult, op1=ALU.add)
                    nc.vector.tensor_reduce(smin[:], score[:], axis=AX.X,
                                            op=ALU.min)
                    nc.vector.tensor_scalar(
                        out=oh[:], in0=score[:], scalar1=smin[:], scalar2=None,
                        op0=ALU.is_equal)
                    nc.vector.tensor_scalar(
                        out=ga[:], in0=nft, scalar1=float(tr), scalar2=None,
                        op0=ALU.is_gt)  # ga = (n > tr)
                    nc.vector.tensor_scalar(
                        out=ohg[:], in0=oh[:], scalar1=ga[:], scalar2=None,
                        op0=ALU.mult)
                    nc.vector.tensor_tensor(sel[:], sel[:], ohg[:], op=ALU.add)
                    nc.vector.tensor_tensor(
                        msp3, S23, ohg[:].unsqueeze(1).broadcast_to([P, K, K]),
                        op=ALU.mult)
                    nc.vector.tensor_reduce(scol[:], msp3, axis=AX.X,
                                            op=ALU.add)
                    nc.vector.tensor_tensor(msim[:], msim[:], scol[:],
                                            op=ALU.max)

                if dbg:
                    nc.sync.dma_start(Ld[tg], L_t[t][:])
                    nc.sync.dma_start(confd[tg], conf[:])
                    nc.sync.dma_start(S2d[tg], S2[:])
                    nc.sync.dma_start(seld[tg], sel[:])
                    nc.sync.dma_start(Td[tg], Sf[:])
                    nc.sync.dma_start(dgd[tg], dg[:])

                u0 = sm.tile([P, K], f32, tag="u0")
                nc.vector.tensor_scalar(
                    out=u0[:], in0=conf[:], scalar1=5.0, scalar2=None,
                    op0=ALU.mult)
                u = sm.tile([P, K], f32, tag="u")
                nc.vector.scalar_tensor_tensor(
                    out=u[:], in0=sel[:], scalar=GBIG, in1=u0[:],
                    op0=ALU.mult, op1=ALU.add)
                gm = sm.tile([P, 1], f32, tag="gm")
                nc.vector.tensor_reduce(gm[:], u[:], axis=AX.X, op=ALU.max,
                                        negate=True)
                eg = sm.tile([P, K], f32, tag="eg")
                zg = sm.tile([P, 1], f32, tag="zg")
                nc.scalar.activation(eg[:], u[:], AF.Exp, bias=gm[:],
                                     accum_out=zg[:])
                rg = sm.tile([P, 1], f32, tag="rg")
                nc.vector.reciprocal(rg[:], zg[:])
                gt = sm.tile([P, K], f32, tag="gt")
                nc.vector.tensor_scalar(
                    out=gt[:], in0=eg[:], scalar1=rg[:], scalar2=None,
                    op0=ALU.mult)
                nc.scalar.dma_start(gout_v[tg], gt[:])

                acc = pools["acc"].tile([P, C], f32, tag="acc")
                nc.vector.tensor_scalar(
                    out=acc[:], in0=L_t[t][:, 0:C], scalar1=gt[:, 0:1],
                    scalar2=None, op0=ALU.mult)
                for k in range(1, K):
                    nc.vector.scalar_tensor_tensor(
                        out=acc[:], in0=L_t[t][:, k * C:(k + 1) * C],
                        scalar=gt[:, k:k + 1], in1=acc[:],
                        op0=ALU.mult, op1=ALU.add)
                nc.scalar.dma_start(out_v[tg], acc[:])

    nc.compile()
    return nc


def _split_bf16(x):
    hi = x.astype(ml_dtypes.bfloat16)
    lo = (x - hi.astype(np.float32)).astype(ml_dtypes.bfloat16)
    return hi, lo


def kernel(z, n_exp, W1, b1, W2, b2):
    from concourse import bass_utils

    key = (BC, N_CORES)
    if key not in _CACHE:
        _CACHE[key] = _build()
    nc = _CACHE[key]

    z = np.asarray(z, dtype=np.float32)
    n_exp = np.asarray(n_exp)
    W1 = np.asarray(W1, dtype=np.float32)
    b1 = np.asarray(b1, dtype=np.float32)
    W2 = np.asarray(W2, dtype=np.float32)
    b2 = np.asarray(b2, dtype=np.float32)

    w1h, w1l = _split_bf16(W1)
    w2h, w2l = _split_bf16(W2)
    b2h, b2l = _split_bf16(b2.reshape(1, K * C))
    b2hl = np.ascontiguousarray(np.concatenate([b2h, b2l], axis=0))
    # b1 rearranged so b1r[p, k*MD+m] = b1[k, m*128+p]
    b1r = np.ascontiguousarray(
        b1.reshape(K, MD, P).transpose(2, 0, 1).reshape(P, K * MD))
    zT = np.ascontiguousarray(z.T)  # [D, B]
    zh, zl = _split_bf16(zT)
    nf = n_exp.astype(np.float32)

    in_maps = []
    for c in range(N_CORES):
        ssl = slice(c * BC, (c + 1) * BC)
        in_maps.append({
            "zth": np.ascontiguousarray(zh[:, ssl]),
            "ztl": np.ascontiguousarray(zl[:, ssl]),
            "nf": np.ascontiguousarray(nf[ssl].reshape(NT, P, 1)),
            "w1h": w1h, "w1l": w1l, "w2h": w2h, "w2l": w2l,
            "b1r": b1r, "b2hl": b2hl,
        })

    trace = bool(int(os.environ.get("BK_TRACE", "0")))
    res = bass_utils.run_bass_kernel_spmd(
        nc, in_maps, core_ids=list(range(N_CORES)), trace=trace)
    kernel.last_result = res

    logits = np.concatenate([res.results[c]["out"] for c in range(N_CORES)], 0)
    gates = np.concatenate([res.results[c]["gout"] for c in range(N_CORES)], 0)
    return logits, gates
